# revision 27
# baseline (speedup 1.0000x reference)
"""4-layer multi-head GAT encoder on 8 Trainium2 NeuronCores (Bass/Tile).

Strategy (dst-sharded message passing):
  - Nodes padded to 10240, split into 80 blocks of 128; core s owns blocks
    [10*s, 10*s+10) (its 1280 "dst" nodes).
  - Per layer, every core computes the full dense projection
    xw_aug = x @ W_aug  (W_aug = [W | W@att_src_h | W@att_dst_h]) and stores
    rows to its own HBM (bf16, row stride 896 = 1792 B):
        xw_hbm[n, 0:768]   = (x W)[n]      (6 heads x 128)
        xw_hbm[n, 768:774] = a_src[n, h]
        xw_hbm[n, 774:780] = a_dst[n, h]   (cols 780:896 zero pad)
    Layers ping-pong between two xw tables so the next layer's dense phase
    is not WAR-serialized against this layer's gathers.
  - Edges (incl. self loops) are sorted by dst and chunked 128-at-a-time per
    dst block (K chunks per block, fixed).  Gathers and edge math are batched
    4 chunks at a time (512 indices per dma_gather call; larger calls
    overflow the SWDGE descriptor ring and hang real silicon):
        feat  = dma_gather(xw_hbm, src_ids)            # [128e, 4, 896] bf16
        adst  = dma_gather(xw_hbm[:, 768:], dst_ids)   # 256 B tail rows
        z     = a_src_e + a_dst_e ; z = max(z, 0.2 z) ; ex = exp(z)
        B     = (col_idx == dst_local)          # one-hot [128e, 4, 128d]
        rhs   = [feat_h * ex_h  for h] ++ [ex]  # [128e, 4, 774], ex bcast
    (one DVE op per group via stride-0 broadcast APs), then per chunk:
        acc  += B_j.T @ rhs_j                   # PSUM accumulate
    Segment softmax denominators land in acc[:, 768:774]; no max-subtraction
    is needed (exp arguments are O(10); any per-segment constant cancels).
  - Block epilogue: out = relu(mean_h(acc_h * recip_h) + bias); layers 0-2
    transpose to channel-major and AllGather across the 8 cores (two half-
    shard collectives per layer so the first overlaps the remaining scatter
    work) so every core has the full x for the next layer's dense phase.
Pad edge slots get dst_local=255 -> all-zero one-hot row -> exact zero
contribution.

Host<->device traffic is minimized (the axon dispatch wall is dominated by
tunnel transfers + program load, not device time):
  - ONE int16 input param per core packs: this core's x shard (bf16 bits,
    channel-major), its 64-row slice of the [512,780] W_aug table (bf16
    bits), dst_local (uint8 bits), bias as hi/lo bf16 pair, and the flat
    src gather-index stream.  x and W are AllGathered on device; the dst
    index stream is REBUILT on device (1280*partition_id + 128*block +
    dst_local); gather indices are replicated into the 8x16-partition
    SWDGE layout by on-device DMAs; col_idx comes from iota; bias rows
    are partition-broadcast.
  - Output is row-quantized uint8 (per-node-row f32 max rides in 128 extra
    rows of the same tensor; host reconstructs f32 = u8 * rowmax/255,
    adding <=0.2% of global absmax error).
  - A persistent JAX compilation cache skips NEFF recompilation+rewrap on
    repeat dispatches.
"""

import os
import numpy as np
import ml_dtypes

import jax

try:
    jax.config.update("jax_compilation_cache_dir", "/root/.cache/jax_bass_cache")
    jax.config.update("jax_persistent_cache_min_compile_time_secs", 0.0)
    jax.config.update("jax_persistent_cache_min_entry_size_bytes", 0)
except Exception:
    pass

import concourse.bass as bass
import concourse.bacc as bacc
import concourse.mybir as mybir
import concourse.tile as tile
from concourse.bass_utils import run_bass_kernel_spmd
from concourse.masks import make_identity

BF16 = mybir.dt.bfloat16
F32 = mybir.dt.float32
I16 = mybir.dt.int16
U8 = mybir.dt.uint8
AF = mybir.ActivationFunctionType
OP = mybir.AluOpType

N = 10000
E = 160000
H = 6
D = 128
L = 4
NEG = 0.2
CORES = 8

NPAD = 10240
NB = NPAD // 128          # 80 node blocks
BPC = NB // CORES         # 10 blocks per core
SHARD = BPC * 128         # 1280 nodes per core
XW_STRIDE = 896           # bf16 row stride of xw_hbm (256B multiple)
RW = 774                  # useful row width: 768 feat + 6 a_src
WCOLS = 780               # dense out: 768 feat + 6 a_src + 6 a_dst
WSH = L * D // CORES      # 64 W_aug rows per core
G_F = 4                   # chunks per gather call / batched-edge-math group


def _pack_offsets(K):
    """Field offsets (in int16 elements) inside the per-core packed param."""
    C = K * BPC
    o_x = 0                      # [128, SHARD] bf16 bits, x shard chan-major
    o_dl = o_x + 128 * SHARD     # [128, C] uint8 bits, dst_local
    o_w = o_dl + 64 * C          # [WSH, WCOLS] bf16 bits, W_aug row slice
    o_b = o_w + WSH * WCOLS      # [L, 256] bf16 bits, bias hi|lo
    o_fi = o_b + L * 256         # [C*128] int16 flat src ids
    o_bs = o_fi + C * 128        # [C] int16 per-chunk local block base 128*t
    tot = o_bs + C
    return C, o_x, o_dl, o_w, o_b, o_fi, o_bs, tot


def _host_prep(x, edge_index, Ws, asrcs, adsts, bs):
    """All numpy preprocessing. Returns dict with packed per-core params."""
    # ---- weights -----------------------------------------------------------
    w_aug = np.zeros((L * D, WCOLS), np.float32)
    for l in range(L):
        W = Ws[l].astype(np.float32)            # [128, 768]
        w_aug[l * D:(l + 1) * D, :768] = W
        Wh = W.reshape(D, H, D)                  # [128, h, 128]
        w_aug[l * D:(l + 1) * D, 768:774] = np.einsum(
            "dhc,hc->dh", Wh, asrcs[l][0])
        w_aug[l * D:(l + 1) * D, 774:780] = np.einsum(
            "dhc,hc->dh", Wh, adsts[l][0])
    w_aug16 = w_aug.astype(ml_dtypes.bfloat16)

    b_all = np.stack([bs[l] for l in range(L)]).astype(np.float32)  # [L,128]
    b_hi = b_all.astype(ml_dtypes.bfloat16)
    b_lo = (b_all - b_hi.astype(np.float32)).astype(ml_dtypes.bfloat16)
    # flat [1, L*256]: per layer, 128 hi then 128 lo
    b_field = np.concatenate([b_hi, b_lo], axis=1).reshape(1, L * 256)

    # ---- x, channel-major padded layout [8*128, SHARD] ---------------------
    xp = np.zeros((NPAD, D), ml_dtypes.bfloat16)
    xp[:N] = x.astype(ml_dtypes.bfloat16)
    # x0T[s*128 + c, t*128 + p] = xp[s*1280 + t*128 + p, c]
    x0t = np.ascontiguousarray(
        xp.reshape(CORES, BPC, 128, D)           # [s, t, p, c]
        .transpose(0, 3, 1, 2)                   # [s, c, t, p]
        .reshape(CORES, 128, SHARD)
    )

    # ---- edges -------------------------------------------------------------
    src = np.concatenate([edge_index[0], np.arange(N, dtype=np.int64)])
    dst = np.concatenate([edge_index[1], np.arange(N, dtype=np.int64)])
    sort_key = dst.astype(np.int16) if N < 2 ** 15 else dst.astype(np.int32)
    order = np.argsort(sort_key, kind="stable")
    src, dst = src[order], dst[order]
    blk = dst // 128
    counts = np.bincount(blk, minlength=NB)
    K = int(max(1, np.max((counts + 127) // 128)))
    C, o_x, o_dl, o_w, o_b, o_fi, o_bs, tot = _pack_offsets(K)

    bounds = np.concatenate([[0], np.cumsum(counts)])
    # slot for edge e: block blk[e], rank within block, laid out flat as
    # [NB, K, 128] -> [CORES, C=BPC*K, 128]
    rank = np.arange(len(src)) - bounds[blk]
    pos = blk * (K * 128) + rank
    src_flat = np.zeros(NB * K * 128, np.int16)
    dl_flat = np.full(NB * K * 128, 255.0, np.float32)
    src_flat[pos] = src
    dl_flat[pos] = (dst - blk * 128).astype(np.float32)
    src_sl = src_flat.reshape(CORES, C, 128)
    base_row = (128 * (np.arange(C) // K)).astype(np.int16)
    # dst_local tile [128, C]: value for (chunk c, lane p) at [p, c]
    dl_tile = np.ascontiguousarray(
        dl_flat.reshape(CORES, C, 128).transpose(0, 2, 1)
    ).astype(np.uint8)

    # ---- pack per-core int16 param ----------------------------------------
    pk = np.zeros((CORES, 1, tot), np.int16)
    for s in range(CORES):
        pk[s, 0, o_x:o_x + 128 * SHARD] = \
            np.ascontiguousarray(x0t[s]).view(np.int16).reshape(-1)
        pk[s, 0, o_dl:o_dl + 64 * C] = \
            np.ascontiguousarray(dl_tile[s]).reshape(-1).view(np.int16)
        pk[s, 0, o_w:o_w + WSH * WCOLS] = np.ascontiguousarray(
            w_aug16[s * WSH:(s + 1) * WSH]).view(np.int16).reshape(-1)
        pk[s, 0, o_b:o_b + L * 256] = \
            np.ascontiguousarray(b_field).view(np.int16).reshape(-1)
        pk[s, 0, o_fi:o_fi + C * 128] = src_sl[s].reshape(-1)
        pk[s, 0, o_bs:o_bs + C] = base_row

    return dict(K=K, pk=pk)


def _build_nc(K):
    C, o_x, o_dl, o_w, o_b, o_fi, o_bs, tot = _pack_offsets(K)
    nc = bacc.Bacc(
        "TRN2", target_bir_lowering=False, debug=False, num_devices=CORES,
    )

    pk_in = nc.declare_dram_parameter("pk", [1, tot], I16, isOutput=False)
    # row-quantized output: u8 values + per-node-row f32 max (host divides
    # by 255); halves the donated-zeros upload and the result download.
    # The f32 scale bits ride in 4*BPC extra rows (transposed: row j holds
    # byte j of every partition's scale) so there is only ONE output array.
    out_ext = nc.declare_dram_parameter("out_shard", [SHARD + 4 * BPC, 128],
                                        U8, isOutput=True)

    def pk_field(off, n, rearr=None, **kw):
        ap = pk_in[0:1, off:off + n]
        if rearr:
            ap = ap.rearrange(rearr, **kw)
        return ap

    with tile.TileContext(nc) as tc:
        with (
            tc.tile_pool(name="dram", bufs=1, space="DRAM") as dram,
            tc.tile_pool(name="const", bufs=1) as constp,
            tc.tile_pool(name="wp", bufs=1) as wp,
            tc.tile_pool(name="lhs", bufs=4) as lhsp,
            tc.tile_pool(name="featg", bufs=4) as featp,
            tc.tile_pool(name="sideg", bufs=4) as sidep,
            tc.tile_pool(name="work", bufs=4) as workp,
            tc.tile_pool(name="ev", bufs=4) as evp,
            tc.tile_pool(name="xt", bufs=1) as xtp,
            tc.tile_pool(name="psd", bufs=2, space="PSUM") as psdp,
            tc.tile_pool(name="acc", bufs=2, space="PSUM") as accp,
        ):
            # ---- persistent DRAM scratch ----------------------------------
            xw_hbms = [dram.tile([NPAD, XW_STRIDE], BF16,
                                 tag=f"xw{i}", name=f"xw_hbm_{i}")
                       for i in range(2)]
            HSH = SHARD // 2
            ag_ins = [dram.tile([128, HSH], BF16, tag=f"agi{l}{h}",
                                name=f"ag_in_{l}_{h}")
                      for l in range(L - 1) for h in range(2)]
            ag_outs = [dram.tile([CORES * 128, HSH], BF16,
                                 addr_space="Shared", tag=f"ago{l}{h}",
                                 name=f"ag_out_{l}_{h}")
                       for l in range(L - 1) for h in range(2)]
            xg_in = dram.tile([128, SHARD], BF16, tag="xgi", name="xg_in")
            xg_out = dram.tile([CORES * 128, SHARD], BF16,
                               addr_space="Shared", tag="xgo", name="xg_out")
            wg_in = dram.tile([WSH, WCOLS], BF16, tag="wgi", name="wg_in")
            w_all = dram.tile([L * D, WCOLS], BF16,
                              addr_space="Shared", tag="wgo", name="w_all")

            # ---- gather x shard + W slice, AllGather across cores ---------
            nc.sync.dma_start(
                out=xg_in[:, :],
                in_=pk_field(o_x, 128 * SHARD, "x (p c) -> (x p) c",
                             p=128).bitcast(BF16))
            nc.gpsimd.collective_compute(
                "AllGather", OP.bypass,
                replica_groups=[list(range(CORES))],
                ins=[xg_in.opt()], outs=[xg_out.opt()],
            )
            nc.sync.dma_start(
                out=wg_in[:, :],
                in_=pk_field(o_w, WSH * WCOLS, "x (p c) -> (x p) c",
                             p=WSH).bitcast(BF16))
            nc.gpsimd.collective_compute(
                "AllGather", OP.bypass,
                replica_groups=[list(range(CORES))],
                ins=[wg_in.opt()], outs=[w_all.opt()],
            )

            # ---- constants into SBUF --------------------------------------
            ci_i16 = constp.tile([128, 128], I16)
            nc.gpsimd.iota(out=ci_i16[:], pattern=[[1, 128]], base=0,
                           channel_multiplier=0)
            col_idx = constp.tile([128, 128], BF16)
            nc.vector.tensor_copy(out=col_idx[:], in_=ci_i16[:])

            dl_u8 = constp.tile([128, C], U8)
            nc.sync.dma_start(
                out=dl_u8[:],
                in_=pk_in[0:1, o_dl:o_dl + 64 * C].bitcast(U8).rearrange(
                    "x (p c) -> (x p) c", p=128))
            dl_sb = constp.tile([128, C], F32)
            nc.vector.tensor_copy(out=dl_sb[:], in_=dl_u8[:])

            # SWDGE index layout: value for flat idx i at [i%16, i//16],
            # replicated across the 8 groups of 16 partitions (per Q7 core)
            fi_sb = constp.tile([128, C * 8], I16)
            si_sb = constp.tile([128, C * 8], I16)
            for k in range(8):
                nc.sync.dma_start(
                    out=fi_sb[16 * k:16 * (k + 1), :],
                    in_=pk_field(o_fi, C * 128, "x (jj i) -> (x i) jj", i=16))
            # dst ids rebuilt on device: 1280*pid + 128*t + min(dst_local,127)
            # (pad lanes clamp to an in-range row; their one-hot row is 0)
            pid_u = constp.tile([1, 1], mybir.dt.uint32)
            nc.sync.dma_start(out=pid_u[:],
                              in_=nc.partition_id_tensor[0:1, 0:1])
            pid_f = constp.tile([1, 1], F32)
            nc.vector.tensor_copy(out=pid_f[:], in_=pid_u[:])
            pid_bc = constp.tile([128, 1], F32)
            nc.gpsimd.partition_broadcast(out_ap=pid_bc[:],
                                          in_ap=pid_f[0:1, :])
            pidm = constp.tile([128, 1], F32)
            nc.vector.tensor_scalar(out=pidm[:], in0=pid_bc[:],
                                    scalar1=float(SHARD), scalar2=None,
                                    op0=OP.mult)
            bs_i = constp.tile([1, C], I16)
            nc.sync.dma_start(out=bs_i[:], in_=pk_in[0:1, o_bs:o_bs + C])
            bs_row = constp.tile([1, C], F32)
            nc.vector.tensor_copy(out=bs_row[:], in_=bs_i[:])
            bs_bc = constp.tile([128, C], F32)
            nc.gpsimd.partition_broadcast(out_ap=bs_bc[:],
                                          in_ap=bs_row[0:1, :])
            dl_min = constp.tile([128, C], F32)
            nc.vector.tensor_scalar(out=dl_min[:], in0=dl_sb[:],
                                    scalar1=127.0, scalar2=None, op0=OP.min)
            si_f = constp.tile([128, C], F32)
            nc.vector.scalar_tensor_tensor(
                out=si_f[:], in0=dl_min[:], scalar=pidm[:, 0:1],
                in1=bs_bc[:], op0=OP.add, op1=OP.add)
            si_i = constp.tile([128, C], I16)
            nc.vector.tensor_copy(out=si_i[:], in_=si_f[:])
            si_dram = dram.tile([128, C], I16, tag="sid", name="si_dram")
            nc.sync.dma_start(out=si_dram[:, :], in_=si_i[:])
            for k in range(8):
                nc.sync.dma_start(
                    out=si_sb[16 * k:16 * (k + 1), :].rearrange(
                        "p (c g) -> p c g", g=8),
                    in_=si_dram[:, :].rearrange("(g p) c -> p c g", p=16))

            # bias: hi/lo bf16 pair -> f32 row on partition 0, broadcast
            b_raw = constp.tile([1, L * 256], BF16)
            nc.sync.dma_start(out=b_raw[:], in_=pk_field(o_b, L * 256)
                              .bitcast(BF16))
            b_bcs = []
            for l in range(L):
                b_sum = constp.tile([1, 128], F32, tag=f"bsum{l}")
                nc.vector.tensor_tensor(
                    out=b_sum[:],
                    in0=b_raw[0:1, l * 256:l * 256 + 128],
                    in1=b_raw[0:1, l * 256 + 128:(l + 1) * 256], op=OP.add)
                b_bc = constp.tile([128, 128], F32, tag=f"bbc{l}")
                nc.gpsimd.partition_broadcast(
                    out_ap=b_bc[:], in_ap=b_sum[0:1, :])
                b_bcs.append(b_bc)

            ident = constp.tile([128, 128], F32)
            make_identity(nc, ident[:])
            ZB = min(10, NB)  # blocks per zero-fill store
            zero_sb = constp.tile([128, ZB, XW_STRIDE - WCOLS], BF16)
            nc.vector.memset(zero_sb[:], 0.0)
            for xw in xw_hbms:
                for b0 in range(0, NB - ZB + 1, ZB):
                    dstz = xw[b0 * 128:(b0 + ZB) * 128, WCOLS:XW_STRIDE]
                    nc.sync.dma_start(
                        out=dstz.rearrange("(j p) c -> p j c", p=128),
                        in_=zero_sb[:])
                rem = NB % ZB
                if rem:
                    b0 = NB - rem
                    dstz = xw[b0 * 128:NB * 128, WCOLS:XW_STRIDE]
                    nc.sync.dma_start(
                        out=dstz.rearrange("(j p) c -> p j c", p=128),
                        in_=zero_sb[:, 0:rem, :])

            for layer in range(L):
                xw_hbm = xw_hbms[layer % 2]

                # ======== dense phase: xw_aug for all 80 blocks ============
                w_sb = wp.tile([128, WCOLS], BF16, tag="wsb")
                nc.sync.dma_start(out=w_sb[:],
                                  in_=w_all[layer * D:(layer + 1) * D, :])
                b_sb = b_bcs[layer]

                GB = BPC // 2  # node-tiles per batched load/store
                for g5 in range(NB // GB):
                    s, half = divmod(g5, BPC // GB)
                    lhsT = lhsp.tile([128, GB * 128], BF16, tag="lhsT")
                    if layer == 0:
                        src_ap = xg_out[s * 128:(s + 1) * 128,
                                        half * GB * 128:(half + 1) * GB * 128]
                    else:
                        src_ap = ag_outs[(layer - 1) * 2 + half][
                            s * 128:(s + 1) * 128, :]
                    nc.sync.dma_start(out=lhsT[:], in_=src_ap)
                    xwrows = evp.tile([128, GB, WCOLS], BF16, tag="xwrow")
                    for tt in range(GB):
                        b = g5 * GB + tt
                        psd = psdp.tile([128, WCOLS], F32, space="PSUM",
                                        tag="psd")
                        nc.tensor.matmul(
                            out=psd[:, 0:512],
                            lhsT=lhsT[:, tt * 128:(tt + 1) * 128],
                            rhs=w_sb[:, 0:512], start=True, stop=True)
                        nc.tensor.matmul(
                            out=psd[:, 512:WCOLS],
                            lhsT=lhsT[:, tt * 128:(tt + 1) * 128],
                            rhs=w_sb[:, 512:WCOLS], start=True, stop=True)
                        if b % 2 == 0:
                            nc.vector.tensor_copy(out=xwrows[:, tt, :],
                                                  in_=psd[:, 0:WCOLS])
                        else:
                            nc.scalar.activation(out=xwrows[:, tt, :],
                                                 in_=psd[:, 0:WCOLS],
                                                 func=AF.Copy)
                    dst = xw_hbm[g5 * GB * 128:(g5 + 1) * GB * 128, 0:WCOLS]
                    nc.sync.dma_start(
                        out=dst.rearrange("(j p) c -> p j c", p=128),
                        in_=xwrows[:])

                # ======== scatter phase: own 10 blocks =====================
                feat_tiles = {}
                grp_tiles = {}
                xt_sb = xtp.tile([128, SHARD], BF16, tag="xt")
                out_acc = (xtp.tile([128, BPC, 128], U8, tag="oacc",
                                    name="out_acc")
                           if layer == L - 1 else None)
                sc_acc = (xtp.tile([128, BPC], F32, tag="oscl",
                                   name="sc_acc")
                          if layer == L - 1 else None)

                for j in range(C):
                    t, jk = divmod(j, K)
                    # group gathers + batched edge math (issued at first use)
                    g = j // G_F
                    if g not in feat_tiles:
                        g0 = g * G_F
                        gn = min(G_F, C - g0)
                        ft = featp.tile([128, gn, XW_STRIDE], BF16, tag="ft")
                        nc.gpsimd.dma_gather(
                            out_ap=ft[:],
                            in_ap=xw_hbm[:, :],
                            idxs_ap=fi_sb[:, g0 * 8:(g0 + gn) * 8],
                            num_idxs=gn * 128,
                            num_idxs_reg=gn * 128,
                            elem_size=XW_STRIDE,
                        )
                        feat_tiles[g] = ft
                        st = sidep.tile([128, gn, 128], BF16, tag="st")
                        nc.gpsimd.dma_gather(
                            out_ap=st[:],
                            in_ap=xw_hbm[:, 768:XW_STRIDE],
                            idxs_ap=si_sb[:, g0 * 8:(g0 + gn) * 8],
                            num_idxs=gn * 128,
                            num_idxs_reg=gn * 128,
                            elem_size=128,
                            elem_step=XW_STRIDE,
                        )
                        # z = a_src[src] + a_dst[dst]; leaky relu; exp --
                        # all gn chunks x 6 heads in one op each
                        z4 = workp.tile([128, gn, 6], F32, tag="z")
                        nc.vector.tensor_tensor(
                            out=z4[:], in0=ft[:, :, 768:774],
                            in1=st[:, :, 6:12], op=OP.add)
                        z24 = workp.tile([128, gn, 6], F32, tag="z2")
                        nc.vector.scalar_tensor_tensor(
                            out=z24[:], in0=z4[:], scalar=NEG, in1=z4[:],
                            op0=OP.mult, op1=OP.max)
                        rhs4 = workp.tile([128, gn, RW], BF16, tag="rhs")
                        nc.scalar.activation(out=rhs4[:, :, 768:774],
                                             in_=z24[:], func=AF.Exp)
                        # rhs feat cols: feat_h * ex_h, broadcast ex over c
                        mul_in0 = ft[:, :, 0:768].rearrange(
                            "p g (h c) -> p g h c", h=H)
                        mul_in1 = rhs4[:, :, 768:774].rearrange(
                            "p g (h c) -> p g h c", c=1)
                        mul_in0, mul_in1 = bass.broadcast_tensor_aps(
                            mul_in0, mul_in1)
                        nc.vector.tensor_tensor(
                            out=rhs4[:, :, 0:768].rearrange(
                                "p g (h c) -> p g h c", h=H),
                            in0=mul_in0, in1=mul_in1, op=OP.mult)
                        # one-hot rows: (col == dst_local), all gn chunks
                        Bt4 = workp.tile([128, gn, 128], BF16, tag="B")
                        oh_in0 = col_idx[:, :].rearrange(
                            "p (g c) -> p g c", g=1)
                        oh_in1 = dl_sb[:, g0:g0 + gn].rearrange(
                            "p (g c) -> p g c", c=1)
                        oh_in0, oh_in1 = bass.broadcast_tensor_aps(
                            oh_in0, oh_in1)
                        nc.vector.tensor_tensor(
                            out=Bt4[:], in0=oh_in0, in1=oh_in1,
                            op=OP.is_equal)
                        grp_tiles[g] = (rhs4, Bt4)
                    jj = j - g * G_F
                    rhs4, Bt4 = grp_tiles[g]
                    rhs = rhs4[:, jj, :]
                    Bt = Bt4[:, jj, :]

                    if jk == 0:
                        acc = accp.tile([128, RW], F32, space="PSUM",
                                        tag="acc")
                        feat_tiles[("acc", t)] = acc
                    acc = feat_tiles[("acc", t)]
                    nc.tensor.matmul(out=acc[:, 0:512], lhsT=Bt[:],
                                     rhs=rhs[:, 0:512],
                                     start=(jk == 0), stop=(jk == K - 1))
                    nc.tensor.matmul(out=acc[:, 512:RW], lhsT=Bt[:],
                                     rhs=rhs[:, 512:RW],
                                     start=(jk == 0), stop=(jk == K - 1))

                    # -------- block epilogue -------------------------------
                    if jk == K - 1:
                        dpe = workp.tile([128, 6], F32, tag="dpe")
                        nc.vector.tensor_scalar(
                            out=dpe[:], in0=acc[:, 768:774], scalar1=1e-20,
                            scalar2=None, op0=OP.add)
                        recip = workp.tile([128, 6], F32, tag="recip")
                        nc.vector.reciprocal(out=recip[:], in_=dpe[:])
                        accsb = workp.tile([128, 128], F32, tag="accsb")
                        nc.vector.tensor_scalar(
                            out=accsb[:], in0=acc[:, 0:128],
                            scalar1=recip[:, 0:1], scalar2=None, op0=OP.mult)
                        for h in range(1, H):
                            nc.vector.scalar_tensor_tensor(
                                out=accsb[:], in0=acc[:, h * 128:(h + 1) * 128],
                                scalar=recip[:, h:h + 1], in1=accsb[:],
                                op0=OP.mult, op1=OP.add)
                        res = workp.tile([128, 128], F32, tag="res")
                        nc.vector.scalar_tensor_tensor(
                            out=res[:], in0=accsb[:], scalar=1.0 / H,
                            in1=b_sb[:], op0=OP.mult, op1=OP.add)
                        nc.vector.tensor_scalar(
                            out=res[:], in0=res[:], scalar1=0.0,
                            scalar2=None, op0=OP.max)
                        if layer == L - 1:
                            rmax = workp.tile([128, 1], F32, tag="rmax")
                            nc.vector.tensor_reduce(
                                out=rmax[:], in_=res[:],
                                axis=mybir.AxisListType.X, op=OP.max)
                            rg = workp.tile([128, 1], F32, tag="rg")
                            nc.vector.tensor_scalar(
                                out=rg[:], in0=rmax[:], scalar1=1e-30,
                                scalar2=None, op0=OP.max)
                            nc.vector.tensor_copy(out=sc_acc[:, t:t + 1],
                                                  in_=rg[:])
                            rinv = workp.tile([128, 1], F32, tag="rinv")
                            nc.vector.reciprocal(out=rinv[:], in_=rg[:])
                            r255 = workp.tile([128, 1], F32, tag="r255")
                            nc.vector.tensor_scalar(
                                out=r255[:], in0=rinv[:], scalar1=255.0,
                                scalar2=None, op0=OP.mult)
                            nc.vector.tensor_scalar(
                                out=out_acc[:, t, :], in0=res[:],
                                scalar1=r255[:, 0:1], scalar2=None,
                                op0=OP.mult)
                            if t == BPC - 1:
                                nc.sync.dma_start(
                                    out=out_ext[0:SHARD, :].rearrange(
                                        "(t p) c -> p t c", p=128),
                                    in_=out_acc[:])
                                nc.sync.dma_start(
                                    out=out_ext[SHARD:SHARD + 4 * BPC, :]
                                    .rearrange("a c -> c a"),
                                    in_=sc_acc[:].bitcast(U8))
                        else:
                            pst = psdp.tile([128, 128], F32, space="PSUM",
                                            tag="psd")
                            nc.tensor.transpose(out=pst[:], in_=res[:],
                                                identity=ident[:])
                            nc.vector.tensor_copy(
                                out=xt_sb[:, t * 128:(t + 1) * 128],
                                in_=pst[:])
                            if t == BPC // 2 - 1 or t == BPC - 1:
                                h = 0 if t == BPC // 2 - 1 else 1
                                hsl = slice(h * (SHARD // 2),
                                            (h + 1) * (SHARD // 2))
                                agi = ag_ins[layer * 2 + h]
                                nc.sync.dma_start(out=agi[:, :],
                                                  in_=xt_sb[:, hsl])
                                nc.gpsimd.collective_compute(
                                    "AllGather",
                                    OP.bypass,
                                    replica_groups=[list(range(CORES))],
                                    ins=[agi.opt()],
                                    outs=[ag_outs[layer * 2 + h].opt()],
                                )

    nc.compile()
    return nc


_NC_CACHE = {}


def _get_nc(K):
    if K not in _NC_CACHE:
        nc = _build_nc(K)
        # The BIR is immutable once compiled, but the jit lowering
        # re-serializes it (~0.14 s) on every dispatch; serve a cached copy.
        bj = nc.to_json_bytes()
        nc.to_json_bytes = lambda: bj
        _NC_CACHE[K] = nc
    return _NC_CACHE[K]


def kernel(**inputs):
    x = np.asarray(inputs["x"], np.float32)
    ei = np.asarray(inputs["edge_index"], np.int64)
    Ws = [np.asarray(inputs[f"W{i}"], np.float32) for i in range(L)]
    asrcs = [np.asarray(inputs[f"att_src{i}"], np.float32) for i in range(L)]
    adsts = [np.asarray(inputs[f"att_dst{i}"], np.float32) for i in range(L)]
    bs = [np.asarray(inputs[f"b{i}"], np.float32) for i in range(L)]

    prep = _host_prep(x, ei, Ws, asrcs, adsts, bs)
    nc = _get_nc(prep["K"])

    in_maps = [{"pk": prep["pk"][s]} for s in range(CORES)]

    res = run_bass_kernel_spmd(nc, in_maps, list(range(CORES)))
    if os.environ.get("GAT_BENCH"):
        import time
        times = []
        for _ in range(int(os.environ.get("GAT_BENCH_ITERS", "14"))):
            t0 = time.time()
            res = run_bass_kernel_spmd(nc, in_maps, list(range(CORES)))
            times.append(time.time() - t0)
        kernel.bench_wall_s = times
    parts = []
    for s in range(CORES):
        raw = np.asarray(res.results[s]["out_shard"])   # [SHARD+4*BPC, 128]
        u8 = raw[:SHARD]
        rmax = np.ascontiguousarray(
            raw[SHARD:SHARD + 4 * BPC, :].T).view(np.float32)    # [128, BPC]
        # node row t*128+p within the shard has scale rmax[p, t] / 255
        scale = np.ascontiguousarray(rmax.T).reshape(SHARD, 1) / 255.0
        parts.append(u8.astype(np.float32) * scale)
    return np.concatenate(parts, axis=0)[:N]


# revision 28
# speedup vs baseline: 1.0228x; 1.0228x over previous
"""4-layer multi-head GAT encoder on 8 Trainium2 NeuronCores (Bass/Tile).

Strategy (dst-sharded message passing):
  - Nodes padded to 10240, split into 80 blocks of 128; core s owns blocks
    [10*s, 10*s+10) (its 1280 "dst" nodes).
  - Per layer, every core computes the full dense projection
    xw_aug = x @ W_aug  (W_aug = [W | W@att_src_h | W@att_dst_h]) and stores
    rows to its own HBM (bf16, row stride 896 = 1792 B):
        xw_hbm[n, 0:768]   = (x W)[n]      (6 heads x 128)
        xw_hbm[n, 768:774] = a_src[n, h]
        xw_hbm[n, 774:780] = a_dst[n, h]   (cols 780:896 zero pad)
    Layers ping-pong between two xw tables so the next layer's dense phase
    is not WAR-serialized against this layer's gathers.
  - Edges (incl. self loops) are sorted by dst and chunked 128-at-a-time per
    dst block (K chunks per block, fixed).  Gathers and edge math are batched
    4 chunks at a time (512 indices per dma_gather call; larger calls
    overflow the SWDGE descriptor ring and hang real silicon):
        feat  = dma_gather(xw_hbm, src_ids)            # [128e, 4, 896] bf16
        adst  = dma_gather(xw_hbm[:, 768:], dst_ids)   # 256 B tail rows
        z     = a_src_e + a_dst_e ; z = max(z, 0.2 z) ; ex = exp(z)
        B     = (col_idx == dst_local)          # one-hot [128e, 4, 128d]
        rhs   = [feat_h * ex_h  for h] ++ [ex]  # [128e, 4, 774], ex bcast
    (one DVE op per group via stride-0 broadcast APs), then per chunk:
        acc  += B_j.T @ rhs_j                   # PSUM accumulate
    Segment softmax denominators land in acc[:, 768:774]; no max-subtraction
    is needed (exp arguments are O(10); any per-segment constant cancels).
  - Block epilogue: out = relu(mean_h(acc_h * recip_h) + bias); layers 0-2
    transpose to channel-major and AllGather across the 8 cores (two half-
    shard collectives per layer so the first overlaps the remaining scatter
    work) so every core has the full x for the next layer's dense phase.
Pad edge slots get dst_local=255 -> all-zero one-hot row -> exact zero
contribution.

Host<->device traffic is minimized (the axon dispatch wall is dominated by
tunnel transfers + program load, not device time):
  - ONE int16 input param per core packs: this core's x shard (bf16 bits,
    channel-major), its 64-row slice of the [512,780] W_aug table (bf16
    bits), dst_local (uint8 bits), bias as hi/lo bf16 pair, and the flat
    src gather-index stream.  x and W are AllGathered on device; the dst
    index stream is REBUILT on device (1280*partition_id + 128*block +
    dst_local); gather indices are replicated into the 8x16-partition
    SWDGE layout by on-device DMAs; col_idx comes from iota; bias rows
    are partition-broadcast.
  - Output is row-quantized uint8 (per-node-row f32 max rides in 4*BPC
    extra transposed rows of the same tensor; host reconstructs
    f32 = u8 * rowmax/255, adding <=0.2% of global absmax error).
  - A persistent JAX compilation cache skips NEFF recompilation+rewrap on
    repeat dispatches.
"""

import os
import numpy as np
import ml_dtypes

import jax

try:
    jax.config.update("jax_compilation_cache_dir", "/root/.cache/jax_bass_cache")
    jax.config.update("jax_persistent_cache_min_compile_time_secs", 0.0)
    jax.config.update("jax_persistent_cache_min_entry_size_bytes", 0)
except Exception:
    pass

import concourse.bass as bass
import concourse.bacc as bacc
import concourse.mybir as mybir
import concourse.tile as tile
from concourse.bass_utils import run_bass_kernel_spmd
from concourse.masks import make_identity

BF16 = mybir.dt.bfloat16
F32 = mybir.dt.float32
I16 = mybir.dt.int16
U8 = mybir.dt.uint8
AF = mybir.ActivationFunctionType
OP = mybir.AluOpType

N = 10000
E = 160000
H = 6
D = 128
L = 4
NEG = 0.2
CORES = 8

NPAD = 10240
NB = NPAD // 128          # 80 node blocks
BPC = NB // CORES         # 10 blocks per core
SHARD = BPC * 128         # 1280 nodes per core
XW_STRIDE = 896           # bf16 row stride of xw_hbm (256B multiple)
RW = 774                  # useful row width: 768 feat + 6 a_src
WCOLS = 780               # dense out: 768 feat + 6 a_src + 6 a_dst
WSH = L * D // CORES      # 64 W_aug rows per core
G_F = 4                   # chunks per gather call / batched-edge-math group


def _pack_offsets(K):
    """Field offsets (in int16 elements) inside the per-core packed param."""
    C = K * BPC
    o_x = 0                      # [128, SHARD] bf16 bits, x shard chan-major
    o_dl = o_x + 128 * SHARD     # [128, C] uint8 bits, dst_local
    o_w = o_dl + 64 * C          # [WSH, WCOLS] bf16 bits, W_aug row slice
    o_b = o_w + WSH * WCOLS      # [L, 256] bf16 bits, bias hi|lo
    o_fi = o_b + L * 256         # [C*128] int16 flat src ids
    o_bs = o_fi + C * 128        # [C] int16 per-chunk local block base 128*t
    tot = o_bs + C
    return C, o_x, o_dl, o_w, o_b, o_fi, o_bs, tot


def _host_prep(x, edge_index, Ws, asrcs, adsts, bs):
    """All numpy preprocessing. Returns dict with packed per-core params."""
    # ---- weights -----------------------------------------------------------
    w_aug = np.zeros((L * D, WCOLS), np.float32)
    for l in range(L):
        W = Ws[l].astype(np.float32)            # [128, 768]
        w_aug[l * D:(l + 1) * D, :768] = W
        Wh = W.reshape(D, H, D)                  # [128, h, 128]
        w_aug[l * D:(l + 1) * D, 768:774] = np.einsum(
            "dhc,hc->dh", Wh, asrcs[l][0])
        w_aug[l * D:(l + 1) * D, 774:780] = np.einsum(
            "dhc,hc->dh", Wh, adsts[l][0])
    w_aug16 = w_aug.astype(ml_dtypes.bfloat16)

    b_all = np.stack([bs[l] for l in range(L)]).astype(np.float32)  # [L,128]
    b_hi = b_all.astype(ml_dtypes.bfloat16)
    b_lo = (b_all - b_hi.astype(np.float32)).astype(ml_dtypes.bfloat16)
    # flat [1, L*256]: per layer, 128 hi then 128 lo
    b_field = np.concatenate([b_hi, b_lo], axis=1).reshape(1, L * 256)

    # ---- x, channel-major padded layout [8*128, SHARD] ---------------------
    xp = np.zeros((NPAD, D), ml_dtypes.bfloat16)
    xp[:N] = x.astype(ml_dtypes.bfloat16)
    # x0T[s*128 + c, t*128 + p] = xp[s*1280 + t*128 + p, c]
    x0t = np.ascontiguousarray(
        xp.reshape(CORES, BPC, 128, D)           # [s, t, p, c]
        .transpose(0, 3, 1, 2)                   # [s, c, t, p]
        .reshape(CORES, 128, SHARD)
    )

    # ---- edges -------------------------------------------------------------
    src = np.concatenate([edge_index[0], np.arange(N, dtype=np.int64)])
    dst = np.concatenate([edge_index[1], np.arange(N, dtype=np.int64)])
    sort_key = dst.astype(np.int16) if N < 2 ** 15 else dst.astype(np.int32)
    order = np.argsort(sort_key, kind="stable")
    src, dst = src[order], dst[order]
    blk = dst // 128
    counts = np.bincount(blk, minlength=NB)
    K = int(max(1, np.max((counts + 127) // 128)))
    C, o_x, o_dl, o_w, o_b, o_fi, o_bs, tot = _pack_offsets(K)

    bounds = np.concatenate([[0], np.cumsum(counts)])
    # slot for edge e: block blk[e], rank within block, laid out flat as
    # [NB, K, 128] -> [CORES, C=BPC*K, 128]
    rank = np.arange(len(src)) - bounds[blk]
    pos = blk * (K * 128) + rank
    src_flat = np.zeros(NB * K * 128, np.int16)
    dl_flat = np.full(NB * K * 128, 255.0, np.float32)
    src_flat[pos] = src
    dl_flat[pos] = (dst - blk * 128).astype(np.float32)
    src_sl = src_flat.reshape(CORES, C, 128)
    base_row = (128 * (np.arange(C) // K)).astype(np.int16)
    # dst_local tile [128, C]: value for (chunk c, lane p) at [p, c]
    dl_tile = np.ascontiguousarray(
        dl_flat.reshape(CORES, C, 128).transpose(0, 2, 1)
    ).astype(np.uint8)

    # ---- pack per-core int16 param ----------------------------------------
    pk = np.zeros((CORES, 1, tot), np.int16)
    for s in range(CORES):
        pk[s, 0, o_x:o_x + 128 * SHARD] = \
            np.ascontiguousarray(x0t[s]).view(np.int16).reshape(-1)
        pk[s, 0, o_dl:o_dl + 64 * C] = \
            np.ascontiguousarray(dl_tile[s]).reshape(-1).view(np.int16)
        pk[s, 0, o_w:o_w + WSH * WCOLS] = np.ascontiguousarray(
            w_aug16[s * WSH:(s + 1) * WSH]).view(np.int16).reshape(-1)
        pk[s, 0, o_b:o_b + L * 256] = \
            np.ascontiguousarray(b_field).view(np.int16).reshape(-1)
        pk[s, 0, o_fi:o_fi + C * 128] = src_sl[s].reshape(-1)
        pk[s, 0, o_bs:o_bs + C] = base_row

    return dict(K=K, pk=pk)


def _build_nc(K):
    C, o_x, o_dl, o_w, o_b, o_fi, o_bs, tot = _pack_offsets(K)
    nc = bacc.Bacc(
        "TRN2", target_bir_lowering=False, debug=False, num_devices=CORES,
    )

    pk_in = nc.declare_dram_parameter("pk", [1, tot], I16, isOutput=False)
    # row-quantized output: u8 values + per-node-row f32 max (host divides
    # by 255); halves the donated-zeros upload and the result download.
    # The f32 scale bits ride in 4*BPC extra rows (transposed: row j holds
    # byte j of every partition's scale) so there is only ONE output array.
    out_ext = nc.declare_dram_parameter("out_shard", [SHARD + 4 * BPC, 128],
                                        U8, isOutput=True)

    def pk_field(off, n, rearr=None, **kw):
        ap = pk_in[0:1, off:off + n]
        if rearr:
            ap = ap.rearrange(rearr, **kw)
        return ap

    with tile.TileContext(nc) as tc:
        with (
            tc.tile_pool(name="dram", bufs=1, space="DRAM") as dram,
            tc.tile_pool(name="const", bufs=1) as constp,
            tc.tile_pool(name="wp", bufs=1) as wp,
            tc.tile_pool(name="lhs", bufs=4) as lhsp,
            tc.tile_pool(name="featg", bufs=4) as featp,
            tc.tile_pool(name="sideg", bufs=4) as sidep,
            tc.tile_pool(name="work", bufs=4) as workp,
            tc.tile_pool(name="ev", bufs=4) as evp,
            tc.tile_pool(name="xt", bufs=1) as xtp,
            tc.tile_pool(name="psd", bufs=2, space="PSUM") as psdp,
            tc.tile_pool(name="acc", bufs=2, space="PSUM") as accp,
        ):
            # ---- persistent DRAM scratch ----------------------------------
            xw_hbms = [dram.tile([NPAD, XW_STRIDE], BF16,
                                 tag=f"xw{i}", name=f"xw_hbm_{i}")
                       for i in range(2)]
            HSH = SHARD // 2
            ag_ins = [dram.tile([128, HSH], BF16, tag=f"agi{l}{h}",
                                name=f"ag_in_{l}_{h}")
                      for l in range(L - 1) for h in range(2)]
            ag_outs = [dram.tile([CORES * 128, HSH], BF16,
                                 addr_space="Shared", tag=f"ago{l}{h}",
                                 name=f"ag_out_{l}_{h}")
                       for l in range(L - 1) for h in range(2)]
            xg_in = dram.tile([128, SHARD], BF16, tag="xgi", name="xg_in")
            xg_out = dram.tile([CORES * 128, SHARD], BF16,
                               addr_space="Shared", tag="xgo", name="xg_out")
            wg_in = dram.tile([WSH, WCOLS], BF16, tag="wgi", name="wg_in")
            w_all = dram.tile([L * D, WCOLS], BF16,
                              addr_space="Shared", tag="wgo", name="w_all")

            # ---- gather x shard + W slice, AllGather across cores ---------
            nc.sync.dma_start(
                out=xg_in[:, :],
                in_=pk_field(o_x, 128 * SHARD, "x (p c) -> (x p) c",
                             p=128).bitcast(BF16))
            nc.gpsimd.collective_compute(
                "AllGather", OP.bypass,
                replica_groups=[list(range(CORES))],
                ins=[xg_in.opt()], outs=[xg_out.opt()],
            )
            nc.sync.dma_start(
                out=wg_in[:, :],
                in_=pk_field(o_w, WSH * WCOLS, "x (p c) -> (x p) c",
                             p=WSH).bitcast(BF16))
            nc.gpsimd.collective_compute(
                "AllGather", OP.bypass,
                replica_groups=[list(range(CORES))],
                ins=[wg_in.opt()], outs=[w_all.opt()],
            )

            # ---- constants into SBUF --------------------------------------
            ci_i16 = constp.tile([128, 128], I16)
            nc.gpsimd.iota(out=ci_i16[:], pattern=[[1, 128]], base=0,
                           channel_multiplier=0)
            col_idx = constp.tile([128, 128], BF16)
            nc.vector.tensor_copy(out=col_idx[:], in_=ci_i16[:])

            dl_u8 = constp.tile([128, C], U8)
            nc.sync.dma_start(
                out=dl_u8[:],
                in_=pk_in[0:1, o_dl:o_dl + 64 * C].bitcast(U8).rearrange(
                    "x (p c) -> (x p) c", p=128))
            dl_sb = constp.tile([128, C], F32)
            nc.vector.tensor_copy(out=dl_sb[:], in_=dl_u8[:])

            # SWDGE index layout: value for flat idx i at [i%16, i//16],
            # replicated across the 8 groups of 16 partitions (per Q7 core)
            fi_sb = constp.tile([128, C * 8], I16)
            si_sb = constp.tile([128, C * 8], I16)
            for k in range(8):
                nc.sync.dma_start(
                    out=fi_sb[16 * k:16 * (k + 1), :],
                    in_=pk_field(o_fi, C * 128, "x (jj i) -> (x i) jj", i=16))
            # dst ids rebuilt on device: 1280*pid + 128*t + min(dst_local,127)
            # (pad lanes clamp to an in-range row; their one-hot row is 0)
            pid_u = constp.tile([1, 1], mybir.dt.uint32)
            nc.sync.dma_start(out=pid_u[:],
                              in_=nc.partition_id_tensor[0:1, 0:1])
            pid_f = constp.tile([1, 1], F32)
            nc.vector.tensor_copy(out=pid_f[:], in_=pid_u[:])
            pid_bc = constp.tile([128, 1], F32)
            nc.gpsimd.partition_broadcast(out_ap=pid_bc[:],
                                          in_ap=pid_f[0:1, :])
            pidm = constp.tile([128, 1], F32)
            nc.vector.tensor_scalar(out=pidm[:], in0=pid_bc[:],
                                    scalar1=float(SHARD), scalar2=None,
                                    op0=OP.mult)
            bs_i = constp.tile([1, C], I16)
            nc.sync.dma_start(out=bs_i[:], in_=pk_in[0:1, o_bs:o_bs + C])
            bs_row = constp.tile([1, C], F32)
            nc.vector.tensor_copy(out=bs_row[:], in_=bs_i[:])
            bs_bc = constp.tile([128, C], F32)
            nc.gpsimd.partition_broadcast(out_ap=bs_bc[:],
                                          in_ap=bs_row[0:1, :])
            dl_min = constp.tile([128, C], F32)
            nc.vector.tensor_scalar(out=dl_min[:], in0=dl_sb[:],
                                    scalar1=127.0, scalar2=None, op0=OP.min)
            si_f = constp.tile([128, C], F32)
            nc.vector.scalar_tensor_tensor(
                out=si_f[:], in0=dl_min[:], scalar=pidm[:, 0:1],
                in1=bs_bc[:], op0=OP.add, op1=OP.add)
            si_i = constp.tile([128, C], I16)
            nc.vector.tensor_copy(out=si_i[:], in_=si_f[:])
            si_dram = dram.tile([128, C], I16, tag="sid", name="si_dram")
            nc.sync.dma_start(out=si_dram[:, :], in_=si_i[:])
            for k in range(8):
                nc.sync.dma_start(
                    out=si_sb[16 * k:16 * (k + 1), :].rearrange(
                        "p (c g) -> p c g", g=8),
                    in_=si_dram[:, :].rearrange("(g p) c -> p c g", p=16))

            # bias: hi/lo bf16 pair -> f32 row on partition 0, broadcast
            b_raw = constp.tile([1, L * 256], BF16)
            nc.sync.dma_start(out=b_raw[:], in_=pk_field(o_b, L * 256)
                              .bitcast(BF16))
            b_bcs = []
            for l in range(L):
                b_sum = constp.tile([1, 128], F32, tag=f"bsum{l}")
                nc.vector.tensor_tensor(
                    out=b_sum[:],
                    in0=b_raw[0:1, l * 256:l * 256 + 128],
                    in1=b_raw[0:1, l * 256 + 128:(l + 1) * 256], op=OP.add)
                b_bc = constp.tile([128, 128], F32, tag=f"bbc{l}")
                nc.gpsimd.partition_broadcast(
                    out_ap=b_bc[:], in_ap=b_sum[0:1, :])
                b_bcs.append(b_bc)

            ident = constp.tile([128, 128], F32)
            make_identity(nc, ident[:])
            ZB = min(10, NB)  # blocks per zero-fill store
            zero_sb = constp.tile([128, ZB, XW_STRIDE - WCOLS], BF16)
            nc.vector.memset(zero_sb[:], 0.0)
            for xw in xw_hbms:
                for b0 in range(0, NB - ZB + 1, ZB):
                    dstz = xw[b0 * 128:(b0 + ZB) * 128, WCOLS:XW_STRIDE]
                    nc.sync.dma_start(
                        out=dstz.rearrange("(j p) c -> p j c", p=128),
                        in_=zero_sb[:])
                rem = NB % ZB
                if rem:
                    b0 = NB - rem
                    dstz = xw[b0 * 128:NB * 128, WCOLS:XW_STRIDE]
                    nc.sync.dma_start(
                        out=dstz.rearrange("(j p) c -> p j c", p=128),
                        in_=zero_sb[:, 0:rem, :])

            for layer in range(L):
                xw_hbm = xw_hbms[layer % 2]

                # ======== dense phase: xw_aug for all 80 blocks ============
                w_sb = wp.tile([128, WCOLS], BF16, tag="wsb")
                nc.sync.dma_start(out=w_sb[:],
                                  in_=w_all[layer * D:(layer + 1) * D, :])
                b_sb = b_bcs[layer]

                GB = BPC // 2  # node-tiles per batched load/store
                for g5 in range(NB // GB):
                    s, half = divmod(g5, BPC // GB)
                    lhsT = lhsp.tile([128, GB * 128], BF16, tag="lhsT")
                    if layer == 0:
                        src_ap = xg_out[s * 128:(s + 1) * 128,
                                        half * GB * 128:(half + 1) * GB * 128]
                    else:
                        src_ap = ag_outs[(layer - 1) * 2 + half][
                            s * 128:(s + 1) * 128, :]
                    nc.sync.dma_start(out=lhsT[:], in_=src_ap)
                    xwrows = evp.tile([128, GB, WCOLS], BF16, tag="xwrow")
                    for tt in range(GB):
                        b = g5 * GB + tt
                        psd = psdp.tile([128, WCOLS], F32, space="PSUM",
                                        tag="psd")
                        nc.tensor.matmul(
                            out=psd[:, 0:512],
                            lhsT=lhsT[:, tt * 128:(tt + 1) * 128],
                            rhs=w_sb[:, 0:512], start=True, stop=True)
                        nc.tensor.matmul(
                            out=psd[:, 512:WCOLS],
                            lhsT=lhsT[:, tt * 128:(tt + 1) * 128],
                            rhs=w_sb[:, 512:WCOLS], start=True, stop=True)
                        if b % 2 == 0:
                            nc.vector.tensor_copy(out=xwrows[:, tt, :],
                                                  in_=psd[:, 0:WCOLS])
                        else:
                            nc.scalar.activation(out=xwrows[:, tt, :],
                                                 in_=psd[:, 0:WCOLS],
                                                 func=AF.Copy)
                    dst = xw_hbm[g5 * GB * 128:(g5 + 1) * GB * 128, 0:WCOLS]
                    nc.sync.dma_start(
                        out=dst.rearrange("(j p) c -> p j c", p=128),
                        in_=xwrows[:])

                # ======== scatter phase: own 10 blocks =====================
                feat_tiles = {}
                grp_tiles = {}
                xt_sb = xtp.tile([128, SHARD], BF16, tag="xt")
                out_acc = (xtp.tile([128, BPC, 128], U8, tag="oacc",
                                    name="out_acc")
                           if layer == L - 1 else None)
                sc_acc = (xtp.tile([128, BPC], F32, tag="oscl",
                                   name="sc_acc")
                          if layer == L - 1 else None)

                for j in range(C):
                    t, jk = divmod(j, K)
                    # group gathers + batched edge math (issued at first use)
                    g = j // G_F
                    if g not in feat_tiles:
                        g0 = g * G_F
                        gn = min(G_F, C - g0)
                        ft = featp.tile([128, gn, XW_STRIDE], BF16, tag="ft")
                        nc.gpsimd.dma_gather(
                            out_ap=ft[:],
                            in_ap=xw_hbm[:, :],
                            idxs_ap=fi_sb[:, g0 * 8:(g0 + gn) * 8],
                            num_idxs=gn * 128,
                            num_idxs_reg=gn * 128,
                            elem_size=XW_STRIDE,
                        )
                        feat_tiles[g] = ft
                        st = sidep.tile([128, gn, 128], BF16, tag="st")
                        nc.gpsimd.dma_gather(
                            out_ap=st[:],
                            in_ap=xw_hbm[:, 768:XW_STRIDE],
                            idxs_ap=si_sb[:, g0 * 8:(g0 + gn) * 8],
                            num_idxs=gn * 128,
                            num_idxs_reg=gn * 128,
                            elem_size=128,
                            elem_step=XW_STRIDE,
                        )
                        # z = a_src[src] + a_dst[dst]; leaky relu; exp --
                        # all gn chunks x 6 heads in one op each
                        z4 = workp.tile([128, gn, 6], F32, tag="z")
                        nc.vector.tensor_tensor(
                            out=z4[:], in0=ft[:, :, 768:774],
                            in1=st[:, :, 6:12], op=OP.add)
                        z24 = workp.tile([128, gn, 6], F32, tag="z2")
                        nc.vector.scalar_tensor_tensor(
                            out=z24[:], in0=z4[:], scalar=NEG, in1=z4[:],
                            op0=OP.mult, op1=OP.max)
                        rhs4 = workp.tile([128, gn, RW], BF16, tag="rhs")
                        nc.scalar.activation(out=rhs4[:, :, 768:774],
                                             in_=z24[:], func=AF.Exp)
                        # rhs feat cols: feat_h * ex_h, broadcast ex over c
                        mul_in0 = ft[:, :, 0:768].rearrange(
                            "p g (h c) -> p g h c", h=H)
                        mul_in1 = rhs4[:, :, 768:774].rearrange(
                            "p g (h c) -> p g h c", c=1)
                        mul_in0, mul_in1 = bass.broadcast_tensor_aps(
                            mul_in0, mul_in1)
                        nc.vector.tensor_tensor(
                            out=rhs4[:, :, 0:768].rearrange(
                                "p g (h c) -> p g h c", h=H),
                            in0=mul_in0, in1=mul_in1, op=OP.mult)
                        # one-hot rows: (col == dst_local), all gn chunks
                        Bt4 = workp.tile([128, gn, 128], BF16, tag="B")
                        oh_in0 = col_idx[:, :].rearrange(
                            "p (g c) -> p g c", g=1)
                        oh_in1 = dl_sb[:, g0:g0 + gn].rearrange(
                            "p (g c) -> p g c", c=1)
                        oh_in0, oh_in1 = bass.broadcast_tensor_aps(
                            oh_in0, oh_in1)
                        nc.vector.tensor_tensor(
                            out=Bt4[:], in0=oh_in0, in1=oh_in1,
                            op=OP.is_equal)
                        grp_tiles[g] = (rhs4, Bt4)
                    jj = j - g * G_F
                    rhs4, Bt4 = grp_tiles[g]
                    rhs = rhs4[:, jj, :]
                    Bt = Bt4[:, jj, :]

                    if jk == 0:
                        acc = accp.tile([128, RW], F32, space="PSUM",
                                        tag="acc")
                        feat_tiles[("acc", t)] = acc
                    acc = feat_tiles[("acc", t)]
                    nc.tensor.matmul(out=acc[:, 0:512], lhsT=Bt[:],
                                     rhs=rhs[:, 0:512],
                                     start=(jk == 0), stop=(jk == K - 1))
                    nc.tensor.matmul(out=acc[:, 512:RW], lhsT=Bt[:],
                                     rhs=rhs[:, 512:RW],
                                     start=(jk == 0), stop=(jk == K - 1))

                    # -------- block epilogue -------------------------------
                    if jk == K - 1:
                        dpe = workp.tile([128, 6], F32, tag="dpe")
                        nc.vector.tensor_scalar(
                            out=dpe[:], in0=acc[:, 768:774], scalar1=1e-20,
                            scalar2=None, op0=OP.add)
                        recip = workp.tile([128, 6], F32, tag="recip")
                        nc.vector.reciprocal(out=recip[:], in_=dpe[:])
                        accsb = workp.tile([128, 128], F32, tag="accsb")
                        nc.vector.tensor_scalar(
                            out=accsb[:], in0=acc[:, 0:128],
                            scalar1=recip[:, 0:1], scalar2=None, op0=OP.mult)
                        for h in range(1, H):
                            nc.vector.scalar_tensor_tensor(
                                out=accsb[:], in0=acc[:, h * 128:(h + 1) * 128],
                                scalar=recip[:, h:h + 1], in1=accsb[:],
                                op0=OP.mult, op1=OP.add)
                        res = workp.tile([128, 128], F32, tag="res")
                        nc.vector.scalar_tensor_tensor(
                            out=res[:], in0=accsb[:], scalar=1.0 / H,
                            in1=b_sb[:], op0=OP.mult, op1=OP.add)
                        nc.vector.tensor_scalar(
                            out=res[:], in0=res[:], scalar1=0.0,
                            scalar2=None, op0=OP.max)
                        if layer == L - 1:
                            rmax = workp.tile([128, 1], F32, tag="rmax")
                            nc.vector.tensor_reduce(
                                out=rmax[:], in_=res[:],
                                axis=mybir.AxisListType.X, op=OP.max)
                            rg = workp.tile([128, 1], F32, tag="rg")
                            nc.vector.tensor_scalar(
                                out=rg[:], in0=rmax[:], scalar1=1e-30,
                                scalar2=None, op0=OP.max)
                            nc.vector.tensor_copy(out=sc_acc[:, t:t + 1],
                                                  in_=rg[:])
                            rinv = workp.tile([128, 1], F32, tag="rinv")
                            nc.vector.reciprocal(out=rinv[:], in_=rg[:])
                            r255 = workp.tile([128, 1], F32, tag="r255")
                            nc.vector.tensor_scalar(
                                out=r255[:], in0=rinv[:], scalar1=255.0,
                                scalar2=None, op0=OP.mult)
                            nc.vector.tensor_scalar(
                                out=out_acc[:, t, :], in0=res[:],
                                scalar1=r255[:, 0:1], scalar2=None,
                                op0=OP.mult)
                            if t == BPC - 1:
                                nc.sync.dma_start(
                                    out=out_ext[0:SHARD, :].rearrange(
                                        "(t p) c -> p t c", p=128),
                                    in_=out_acc[:])
                                nc.sync.dma_start(
                                    out=out_ext[SHARD:SHARD + 4 * BPC, :]
                                    .rearrange("a c -> c a"),
                                    in_=sc_acc[:].bitcast(U8))
                        else:
                            pst = psdp.tile([128, 128], F32, space="PSUM",
                                            tag="psd")
                            nc.tensor.transpose(out=pst[:], in_=res[:],
                                                identity=ident[:])
                            nc.vector.tensor_copy(
                                out=xt_sb[:, t * 128:(t + 1) * 128],
                                in_=pst[:])
                            if t == BPC // 2 - 1 or t == BPC - 1:
                                h = 0 if t == BPC // 2 - 1 else 1
                                hsl = slice(h * (SHARD // 2),
                                            (h + 1) * (SHARD // 2))
                                agi = ag_ins[layer * 2 + h]
                                nc.sync.dma_start(out=agi[:, :],
                                                  in_=xt_sb[:, hsl])
                                nc.gpsimd.collective_compute(
                                    "AllGather",
                                    OP.bypass,
                                    replica_groups=[list(range(CORES))],
                                    ins=[agi.opt()],
                                    outs=[ag_outs[layer * 2 + h].opt()],
                                )

    nc.compile()
    return nc


_NC_CACHE = {}


def _get_nc(K):
    if K not in _NC_CACHE:
        nc = _build_nc(K)
        # The BIR is immutable once compiled, but the jit lowering
        # re-serializes it (~0.14 s) on every dispatch; serve a cached copy.
        bj = nc.to_json_bytes()
        nc.to_json_bytes = lambda: bj
        _NC_CACHE[K] = nc
    return _NC_CACHE[K]


def kernel(**inputs):
    x = np.asarray(inputs["x"], np.float32)
    ei = np.asarray(inputs["edge_index"], np.int64)
    Ws = [np.asarray(inputs[f"W{i}"], np.float32) for i in range(L)]
    asrcs = [np.asarray(inputs[f"att_src{i}"], np.float32) for i in range(L)]
    adsts = [np.asarray(inputs[f"att_dst{i}"], np.float32) for i in range(L)]
    bs = [np.asarray(inputs[f"b{i}"], np.float32) for i in range(L)]

    prep = _host_prep(x, ei, Ws, asrcs, adsts, bs)
    nc = _get_nc(prep["K"])

    in_maps = [{"pk": prep["pk"][s]} for s in range(CORES)]

    res = run_bass_kernel_spmd(nc, in_maps, list(range(CORES)))
    if os.environ.get("GAT_BENCH"):
        import time
        times = []
        for _ in range(int(os.environ.get("GAT_BENCH_ITERS", "14"))):
            t0 = time.time()
            res = run_bass_kernel_spmd(nc, in_maps, list(range(CORES)))
            times.append(time.time() - t0)
        kernel.bench_wall_s = times
    parts = []
    for s in range(CORES):
        raw = np.asarray(res.results[s]["out_shard"])   # [SHARD+4*BPC, 128]
        u8 = raw[:SHARD]
        rmax = np.ascontiguousarray(
            raw[SHARD:SHARD + 4 * BPC, :].T).view(np.float32)    # [128, BPC]
        # node row t*128+p within the shard has scale rmax[p, t] / 255
        scale = np.ascontiguousarray(rmax.T).reshape(SHARD, 1) / 255.0
        parts.append(u8.astype(np.float32) * scale)
    return np.concatenate(parts, axis=0)[:N]


# revision 33
# speedup vs baseline: 1.0956x; 1.0712x over previous
"""4-layer multi-head GAT encoder on 8 Trainium2 NeuronCores (Bass/Tile).

Strategy (dst-sharded message passing):
  - Nodes padded to 10240, split into 80 blocks of 128; core s owns blocks
    [10*s, 10*s+10) (its 1280 "dst" nodes).
  - Per layer, every core computes the full dense projection
    xw_aug = x @ W_aug  (W_aug = [W | W@att_src_h | W@att_dst_h]) and stores
    rows to its own HBM (bf16, row stride 896 = 1792 B):
        xw_hbm[n, 0:768]   = (x W)[n]      (6 heads x 128)
        xw_hbm[n, 768:774] = a_src[n, h]
        xw_hbm[n, 774:780] = a_dst[n, h]   (cols 780:896 zero pad)
    Layers ping-pong between two xw tables so the next layer's dense phase
    is not WAR-serialized against this layer's gathers.
  - Edges (incl. self loops) are sorted by dst and chunked 128-at-a-time per
    dst block (K chunks per block, fixed).  Gathers and edge math are batched
    4 chunks at a time (512 indices per dma_gather call; larger calls
    overflow the SWDGE descriptor ring and hang real silicon):
        feat  = dma_gather(xw_hbm, src_ids)            # [128e, 4, 896] bf16
        adst  = dma_gather(xw_hbm[:, 768:], dst_ids)   # 256 B tail rows
        z     = a_src_e + a_dst_e ; z = max(z, 0.2 z) ; ex = exp(z)
        B     = (col_idx == dst_local)          # one-hot [128e, 4, 128d]
        rhs   = [feat_h * ex_h  for h] ++ [ex]  # [128e, 4, 774], ex bcast
    (one DVE op per group via stride-0 broadcast APs), then per chunk:
        acc  += B_j.T @ rhs_j                   # PSUM accumulate
    Segment softmax denominators land in acc[:, 768:774]; no max-subtraction
    is needed (exp arguments are O(10); any per-segment constant cancels).
  - Block epilogue: out = relu(mean_h(acc_h * recip_h) + bias); layers 0-2
    transpose to channel-major and AllGather across the 8 cores (two half-
    shard collectives per layer so the first overlaps the remaining scatter
    work) so every core has the full x for the next layer's dense phase.
Pad edge slots get dst_local=255 -> all-zero one-hot row -> exact zero
contribution.

Host<->device traffic is minimized (the axon dispatch wall is dominated by
tunnel transfers + program load, not device time):
  - ONE int16 input param per core packs: this core's x shard (bf16 bits,
    channel-major), its 64-row slice of the [512,780] W_aug table (bf16
    bits), dst_local (uint8 bits), bias as hi/lo bf16 pair, and the flat
    src gather-index stream.  x and W are AllGathered on device; the dst
    index stream is REBUILT on device (1280*partition_id + 128*block +
    dst_local); gather indices are replicated into the 8x16-partition
    SWDGE layout by on-device DMAs; col_idx comes from iota; bias rows
    are partition-broadcast.
  - Output is row-quantized uint8 (per-node-row f32 max rides in 4*BPC
    extra transposed rows of the same tensor; host reconstructs
    f32 = u8 * rowmax/255, adding <=0.2% of global absmax error).
  - A persistent JAX compilation cache skips NEFF recompilation+rewrap on
    repeat dispatches.
"""

import os
import numpy as np
import ml_dtypes

import jax

try:
    jax.config.update("jax_compilation_cache_dir", "/root/.cache/jax_bass_cache")
    jax.config.update("jax_persistent_cache_min_compile_time_secs", 0.0)
    jax.config.update("jax_persistent_cache_min_entry_size_bytes", 0)
except Exception:
    pass

import concourse.bass as bass
import concourse.bacc as bacc
import concourse.mybir as mybir
import concourse.tile as tile
from concourse.bass_utils import run_bass_kernel_spmd
from concourse.masks import make_identity

BF16 = mybir.dt.bfloat16
F32 = mybir.dt.float32
I16 = mybir.dt.int16
U8 = mybir.dt.uint8
AF = mybir.ActivationFunctionType
OP = mybir.AluOpType

N = 10000
E = 160000
H = 6
D = 128
L = 4
NEG = 0.2
CORES = 8

NPAD = 10240
NB = NPAD // 128          # 80 node blocks
BPC = NB // CORES         # 10 blocks per core
SHARD = BPC * 128         # 1280 nodes per core
XW_STRIDE = 896           # bf16 row stride of xw_hbm (256B multiple)
RW = 774                  # useful row width: 768 feat + 6 a_src
WCOLS = 780               # dense out: 768 feat + 6 a_src + 6 a_dst
WSH = L * D // CORES      # 64 W_aug rows per core
G_F = 4                   # chunks per gather call / batched-edge-math group


def _pack_offsets(K):
    """Field offsets (in int16 elements) inside the per-core packed param."""
    C = K * BPC
    o_xh = 0                     # [128, SHARD] u8: high 8 of 12-bit x, chan-major
    o_xl = o_xh + 64 * SHARD     # [128, SHARD/2] u8: packed low nibbles
    o_xs = o_xl + 32 * SHARD     # [128] f32 bits: per-channel scale am/2047
    o_dl = o_xs + 256            # [128, C] uint8 bits, dst_local
    o_w = o_dl + 64 * C          # [WSH, WCOLS] bf16 bits, W_aug row slice
    o_b = o_w + WSH * WCOLS      # [L, 256] bf16 bits, bias hi|lo
    o_fi = o_b + L * 256         # [C*128] int16 flat src ids
    o_bs = o_fi + C * 128        # [C] int16 per-chunk local block base 128*t
    tot = o_bs + C
    return C, o_xh, o_xl, o_xs, o_dl, o_w, o_b, o_fi, o_bs, tot


def _host_prep(x, edge_index, Ws, asrcs, adsts, bs):
    """All numpy preprocessing. Returns dict with packed per-core params."""
    # ---- weights -----------------------------------------------------------
    w_aug = np.zeros((L * D, WCOLS), np.float32)
    for l in range(L):
        W = Ws[l].astype(np.float32)            # [128, 768]
        w_aug[l * D:(l + 1) * D, :768] = W
        Wh = W.reshape(D, H, D)                  # [128, h, 128]
        w_aug[l * D:(l + 1) * D, 768:774] = np.einsum(
            "dhc,hc->dh", Wh, asrcs[l][0])
        w_aug[l * D:(l + 1) * D, 774:780] = np.einsum(
            "dhc,hc->dh", Wh, adsts[l][0])
    w_aug16 = w_aug.astype(ml_dtypes.bfloat16)

    b_all = np.stack([bs[l] for l in range(L)]).astype(np.float32)  # [L,128]
    b_hi = b_all.astype(ml_dtypes.bfloat16)
    b_lo = (b_all - b_hi.astype(np.float32)).astype(ml_dtypes.bfloat16)
    # flat [1, L*256]: per layer, 128 hi then 128 lo
    b_field = np.concatenate([b_hi, b_lo], axis=1).reshape(1, L * 256)

    # ---- x, channel-major padded layout, 12-bit per-channel quantized ------
    xp = np.zeros((NPAD, D), np.float32)
    xp[:N] = x
    # x0T[s*128 + c, t*128 + p] = xp[s*1280 + t*128 + p, c]
    x0t = np.ascontiguousarray(
        xp.reshape(CORES, BPC, 128, D)           # [s, t, p, c]
        .transpose(0, 3, 1, 2)                   # [s, c, t, p]
        .reshape(CORES, 128, SHARD)
    )
    am = np.maximum(np.abs(x0t).max(axis=2), 1e-30)          # [CORES, 128]
    u12 = np.clip(np.round(x0t / am[:, :, None] * 2047.0) + 2048.0,
                  1, 4095).astype(np.uint16)
    x_hi = (u12 >> 4).astype(np.uint8)                       # [CORES,128,SHARD]
    lo4 = (u12 & 15).astype(np.uint8)
    HS = SHARD // 2
    x_lo = (lo4[:, :, :HS] | (lo4[:, :, HS:] << 4))          # [CORES,128,HS]
    x_sc = (am / 2047.0).astype(np.float32)                  # [CORES, 128]

    # ---- edges -------------------------------------------------------------
    src = np.concatenate([edge_index[0], np.arange(N, dtype=np.int64)])
    dst = np.concatenate([edge_index[1], np.arange(N, dtype=np.int64)])
    sort_key = dst.astype(np.int16) if N < 2 ** 15 else dst.astype(np.int32)
    order = np.argsort(sort_key, kind="stable")
    src, dst = src[order], dst[order]
    blk = dst // 128
    counts = np.bincount(blk, minlength=NB)
    K = int(max(1, np.max((counts + 127) // 128)))
    C, o_xh, o_xl, o_xs, o_dl, o_w, o_b, o_fi, o_bs, tot = _pack_offsets(K)

    bounds = np.concatenate([[0], np.cumsum(counts)])
    # slot for edge e: block blk[e], rank within block, laid out flat as
    # [NB, K, 128] -> [CORES, C=BPC*K, 128]
    rank = np.arange(len(src)) - bounds[blk]
    pos = blk * (K * 128) + rank
    src_flat = np.zeros(NB * K * 128, np.int16)
    dl_flat = np.full(NB * K * 128, 255.0, np.float32)
    src_flat[pos] = src
    dl_flat[pos] = (dst - blk * 128).astype(np.float32)
    src_sl = src_flat.reshape(CORES, C, 128)
    base_row = (128 * (np.arange(C) // K)).astype(np.int16)
    # dst_local tile [128, C]: value for (chunk c, lane p) at [p, c]
    dl_tile = np.ascontiguousarray(
        dl_flat.reshape(CORES, C, 128).transpose(0, 2, 1)
    ).astype(np.uint8)

    # ---- pack per-core int16 param ----------------------------------------
    pk = np.zeros((CORES, 1, tot), np.int16)
    for s in range(CORES):
        pk[s, 0, o_xh:o_xh + 64 * SHARD] = \
            np.ascontiguousarray(x_hi[s]).reshape(-1).view(np.int16)
        pk[s, 0, o_xl:o_xl + 32 * SHARD] = \
            np.ascontiguousarray(x_lo[s]).reshape(-1).view(np.int16)
        pk[s, 0, o_xs:o_xs + 256] = \
            np.ascontiguousarray(x_sc[s]).view(np.int16).reshape(-1)
        pk[s, 0, o_dl:o_dl + 64 * C] = \
            np.ascontiguousarray(dl_tile[s]).reshape(-1).view(np.int16)
        pk[s, 0, o_w:o_w + WSH * WCOLS] = np.ascontiguousarray(
            w_aug16[s * WSH:(s + 1) * WSH]).view(np.int16).reshape(-1)
        pk[s, 0, o_b:o_b + L * 256] = \
            np.ascontiguousarray(b_field).view(np.int16).reshape(-1)
        pk[s, 0, o_fi:o_fi + C * 128] = src_sl[s].reshape(-1)
        pk[s, 0, o_bs:o_bs + C] = base_row

    return dict(K=K, pk=pk)


def _build_nc(K):
    C, o_xh, o_xl, o_xs, o_dl, o_w, o_b, o_fi, o_bs, tot = _pack_offsets(K)
    nc = bacc.Bacc(
        "TRN2", target_bir_lowering=False, debug=False, num_devices=CORES,
    )

    pk_in = nc.declare_dram_parameter("pk", [1, tot], I16, isOutput=False)
    # row-quantized output: u8 values + per-node-row f32 max (host divides
    # by 255); halves the donated-zeros upload and the result download.
    # The f32 scale bits ride in 4*BPC extra rows (transposed: row j holds
    # byte j of every partition's scale) so there is only ONE output array.
    out_ext = nc.declare_dram_parameter("out_shard", [SHARD + 4 * BPC, 128],
                                        U8, isOutput=True)

    def pk_field(off, n, rearr=None, **kw):
        ap = pk_in[0:1, off:off + n]
        if rearr:
            ap = ap.rearrange(rearr, **kw)
        return ap

    with tile.TileContext(nc) as tc:
        with (
            tc.tile_pool(name="dram", bufs=1, space="DRAM") as dram,
            tc.tile_pool(name="const", bufs=1) as constp,
            tc.tile_pool(name="wp", bufs=1) as wp,
            tc.tile_pool(name="lhs", bufs=4) as lhsp,
            tc.tile_pool(name="featg", bufs=4) as featp,
            tc.tile_pool(name="sideg", bufs=4) as sidep,
            tc.tile_pool(name="work", bufs=4) as workp,
            tc.tile_pool(name="ev", bufs=4) as evp,
            tc.tile_pool(name="xt", bufs=1) as xtp,
            tc.tile_pool(name="psd", bufs=2, space="PSUM") as psdp,
            tc.tile_pool(name="acc", bufs=2, space="PSUM") as accp,
        ):
            # ---- persistent DRAM scratch ----------------------------------
            xw_hbms = [dram.tile([NPAD, XW_STRIDE], BF16,
                                 tag=f"xw{i}", name=f"xw_hbm_{i}")
                       for i in range(2)]
            HSH = SHARD // 2
            ag_ins = [dram.tile([128, HSH], BF16, tag=f"agi{l}{h}",
                                name=f"ag_in_{l}_{h}")
                      for l in range(L - 1) for h in range(2)]
            ag_outs = [dram.tile([CORES * 128, HSH], BF16,
                                 addr_space="Shared", tag=f"ago{l}{h}",
                                 name=f"ag_out_{l}_{h}")
                       for l in range(L - 1) for h in range(2)]
            xg_in = dram.tile([128, SHARD], BF16, tag="xgi", name="xg_in")
            xg_out = dram.tile([CORES * 128, SHARD], BF16,
                               addr_space="Shared", tag="xgo", name="xg_out")
            wg_in = dram.tile([WSH, WCOLS], BF16, tag="wgi", name="wg_in")
            w_all = dram.tile([L * D, WCOLS], BF16,
                              addr_space="Shared", tag="wgo", name="w_all")

            # ---- unpack 12-bit x shard, AllGather across cores ------------
            HS = SHARD // 2
            xhi_u = constp.tile([128, SHARD], U8)
            nc.sync.dma_start(
                out=xhi_u[:],
                in_=pk_in[0:1, o_xh:o_xh + 64 * SHARD].bitcast(U8)
                .rearrange("x (p c) -> (x p) c", p=128))
            xlo_u = constp.tile([128, HS], U8)
            nc.sync.dma_start(
                out=xlo_u[:],
                in_=pk_in[0:1, o_xl:o_xl + 32 * SHARD].bitcast(U8)
                .rearrange("x (p c) -> (x p) c", p=128))
            xs_sb = constp.tile([128, 1], F32)
            nc.sync.dma_start(
                out=xs_sb[:],
                in_=pk_in[0:1, o_xs:o_xs + 256].bitcast(F32)
                .rearrange("x (p c) -> (x p) c", p=128))
            xhi_f = constp.tile([128, SHARD], F32)
            nc.vector.tensor_copy(out=xhi_f[:], in_=xhi_u[:])
            xlo_f = constp.tile([128, HS], F32)
            nc.vector.tensor_copy(out=xlo_f[:], in_=xlo_u[:])
            nlo_u = constp.tile([128, HS], U8)
            nc.vector.tensor_scalar(out=nlo_u[:], in0=xlo_u[:],
                                    scalar1=15, scalar2=None,
                                    op0=OP.bitwise_and)
            nhi_u = constp.tile([128, HS], U8)
            nc.vector.tensor_scalar(out=nhi_u[:], in0=xlo_u[:],
                                    scalar1=4, scalar2=None,
                                    op0=OP.logical_shift_right)
            nib_lo = constp.tile([128, HS], F32)
            nc.vector.tensor_copy(out=nib_lo[:], in_=nlo_u[:])
            nib_hi = constp.tile([128, HS], F32)
            nc.vector.tensor_copy(out=nib_hi[:], in_=nhi_u[:])
            xq = constp.tile([128, SHARD], F32)
            nc.vector.scalar_tensor_tensor(
                out=xq[:, 0:HS], in0=xhi_f[:, 0:HS], scalar=16.0,
                in1=nib_lo[:], op0=OP.mult, op1=OP.add)
            nc.vector.scalar_tensor_tensor(
                out=xq[:, HS:SHARD], in0=xhi_f[:, HS:SHARD], scalar=16.0,
                in1=nib_hi[:], op0=OP.mult, op1=OP.add)
            xctr = constp.tile([128, SHARD], F32)
            nc.vector.tensor_scalar(out=xctr[:], in0=xq[:],
                                    scalar1=2048.0, scalar2=None,
                                    op0=OP.subtract)
            xbf = constp.tile([128, SHARD], BF16)
            nc.vector.tensor_scalar(out=xbf[:], in0=xctr[:],
                                    scalar1=xs_sb[:, 0:1], scalar2=None,
                                    op0=OP.mult)
            nc.sync.dma_start(out=xg_in[:, :], in_=xbf[:])
            nc.gpsimd.collective_compute(
                "AllGather", OP.bypass,
                replica_groups=[list(range(CORES))],
                ins=[xg_in.opt()], outs=[xg_out.opt()],
            )
            nc.sync.dma_start(
                out=wg_in[:, :],
                in_=pk_field(o_w, WSH * WCOLS, "x (p c) -> (x p) c",
                             p=WSH).bitcast(BF16))
            nc.gpsimd.collective_compute(
                "AllGather", OP.bypass,
                replica_groups=[list(range(CORES))],
                ins=[wg_in.opt()], outs=[w_all.opt()],
            )

            # ---- constants into SBUF --------------------------------------
            ci_i16 = constp.tile([128, 128], I16)
            nc.gpsimd.iota(out=ci_i16[:], pattern=[[1, 128]], base=0,
                           channel_multiplier=0)
            col_idx = constp.tile([128, 128], BF16)
            nc.vector.tensor_copy(out=col_idx[:], in_=ci_i16[:])

            dl_u8 = constp.tile([128, C], U8)
            nc.sync.dma_start(
                out=dl_u8[:],
                in_=pk_in[0:1, o_dl:o_dl + 64 * C].bitcast(U8).rearrange(
                    "x (p c) -> (x p) c", p=128))
            dl_sb = constp.tile([128, C], F32)
            nc.vector.tensor_copy(out=dl_sb[:], in_=dl_u8[:])

            # SWDGE index layout: value for flat idx i at [i%16, i//16],
            # replicated across the 8 groups of 16 partitions (per Q7 core)
            fi_sb = constp.tile([128, C * 8], I16)
            si_sb = constp.tile([128, C * 8], I16)
            for k in range(8):
                nc.sync.dma_start(
                    out=fi_sb[16 * k:16 * (k + 1), :],
                    in_=pk_field(o_fi, C * 128, "x (jj i) -> (x i) jj", i=16))
            # dst ids rebuilt on device: 1280*pid + 128*t + min(dst_local,127)
            # (pad lanes clamp to an in-range row; their one-hot row is 0)
            pid_u = constp.tile([1, 1], mybir.dt.uint32)
            nc.sync.dma_start(out=pid_u[:],
                              in_=nc.partition_id_tensor[0:1, 0:1])
            pid_f = constp.tile([1, 1], F32)
            nc.vector.tensor_copy(out=pid_f[:], in_=pid_u[:])
            pid_bc = constp.tile([128, 1], F32)
            nc.gpsimd.partition_broadcast(out_ap=pid_bc[:],
                                          in_ap=pid_f[0:1, :])
            pidm = constp.tile([128, 1], F32)
            nc.vector.tensor_scalar(out=pidm[:], in0=pid_bc[:],
                                    scalar1=float(SHARD), scalar2=None,
                                    op0=OP.mult)
            bs_i = constp.tile([1, C], I16)
            nc.sync.dma_start(out=bs_i[:], in_=pk_in[0:1, o_bs:o_bs + C])
            bs_row = constp.tile([1, C], F32)
            nc.vector.tensor_copy(out=bs_row[:], in_=bs_i[:])
            bs_bc = constp.tile([128, C], F32)
            nc.gpsimd.partition_broadcast(out_ap=bs_bc[:],
                                          in_ap=bs_row[0:1, :])
            dl_min = constp.tile([128, C], F32)
            nc.vector.tensor_scalar(out=dl_min[:], in0=dl_sb[:],
                                    scalar1=127.0, scalar2=None, op0=OP.min)
            si_f = constp.tile([128, C], F32)
            nc.vector.scalar_tensor_tensor(
                out=si_f[:], in0=dl_min[:], scalar=pidm[:, 0:1],
                in1=bs_bc[:], op0=OP.add, op1=OP.add)
            si_i = constp.tile([128, C], I16)
            nc.vector.tensor_copy(out=si_i[:], in_=si_f[:])
            si_dram = dram.tile([128, C], I16, tag="sid", name="si_dram")
            nc.sync.dma_start(out=si_dram[:, :], in_=si_i[:])
            for k in range(8):
                nc.sync.dma_start(
                    out=si_sb[16 * k:16 * (k + 1), :].rearrange(
                        "p (c g) -> p c g", g=8),
                    in_=si_dram[:, :].rearrange("(g p) c -> p c g", p=16))

            # bias: hi/lo bf16 pair -> f32 row on partition 0, broadcast
            b_raw = constp.tile([1, L * 256], BF16)
            nc.sync.dma_start(out=b_raw[:], in_=pk_field(o_b, L * 256)
                              .bitcast(BF16))
            b_bcs = []
            for l in range(L):
                b_sum = constp.tile([1, 128], F32, tag=f"bsum{l}")
                nc.vector.tensor_tensor(
                    out=b_sum[:],
                    in0=b_raw[0:1, l * 256:l * 256 + 128],
                    in1=b_raw[0:1, l * 256 + 128:(l + 1) * 256], op=OP.add)
                b_bc = constp.tile([128, 128], F32, tag=f"bbc{l}")
                nc.gpsimd.partition_broadcast(
                    out_ap=b_bc[:], in_ap=b_sum[0:1, :])
                b_bcs.append(b_bc)

            ident = constp.tile([128, 128], F32)
            make_identity(nc, ident[:])
            ZB = min(10, NB)  # blocks per zero-fill store
            zero_sb = constp.tile([128, ZB, XW_STRIDE - WCOLS], BF16)
            nc.vector.memset(zero_sb[:], 0.0)
            for xw in xw_hbms:
                for b0 in range(0, NB - ZB + 1, ZB):
                    dstz = xw[b0 * 128:(b0 + ZB) * 128, WCOLS:XW_STRIDE]
                    nc.sync.dma_start(
                        out=dstz.rearrange("(j p) c -> p j c", p=128),
                        in_=zero_sb[:])
                rem = NB % ZB
                if rem:
                    b0 = NB - rem
                    dstz = xw[b0 * 128:NB * 128, WCOLS:XW_STRIDE]
                    nc.sync.dma_start(
                        out=dstz.rearrange("(j p) c -> p j c", p=128),
                        in_=zero_sb[:, 0:rem, :])

            for layer in range(L):
                xw_hbm = xw_hbms[layer % 2]

                # ======== dense phase: xw_aug for all 80 blocks ============
                w_sb = wp.tile([128, WCOLS], BF16, tag="wsb")
                nc.sync.dma_start(out=w_sb[:],
                                  in_=w_all[layer * D:(layer + 1) * D, :])
                b_sb = b_bcs[layer]

                GB = BPC // 2  # node-tiles per batched load/store
                for g5 in range(NB // GB):
                    s, half = divmod(g5, BPC // GB)
                    lhsT = lhsp.tile([128, GB * 128], BF16, tag="lhsT")
                    if layer == 0:
                        src_ap = xg_out[s * 128:(s + 1) * 128,
                                        half * GB * 128:(half + 1) * GB * 128]
                    else:
                        src_ap = ag_outs[(layer - 1) * 2 + half][
                            s * 128:(s + 1) * 128, :]
                    nc.sync.dma_start(out=lhsT[:], in_=src_ap)
                    xwrows = evp.tile([128, GB, WCOLS], BF16, tag="xwrow")
                    for tt in range(GB):
                        b = g5 * GB + tt
                        psd = psdp.tile([128, WCOLS], F32, space="PSUM",
                                        tag="psd")
                        nc.tensor.matmul(
                            out=psd[:, 0:512],
                            lhsT=lhsT[:, tt * 128:(tt + 1) * 128],
                            rhs=w_sb[:, 0:512], start=True, stop=True)
                        nc.tensor.matmul(
                            out=psd[:, 512:WCOLS],
                            lhsT=lhsT[:, tt * 128:(tt + 1) * 128],
                            rhs=w_sb[:, 512:WCOLS], start=True, stop=True)
                        if b % 2 == 0:
                            nc.vector.tensor_copy(out=xwrows[:, tt, :],
                                                  in_=psd[:, 0:WCOLS])
                        else:
                            nc.scalar.activation(out=xwrows[:, tt, :],
                                                 in_=psd[:, 0:WCOLS],
                                                 func=AF.Copy)
                    dst = xw_hbm[g5 * GB * 128:(g5 + 1) * GB * 128, 0:WCOLS]
                    nc.sync.dma_start(
                        out=dst.rearrange("(j p) c -> p j c", p=128),
                        in_=xwrows[:])

                # ======== scatter phase: own 10 blocks =====================
                feat_tiles = {}
                grp_tiles = {}
                xt_sb = xtp.tile([128, SHARD], BF16, tag="xt")
                out_acc = (xtp.tile([128, BPC, 128], U8, tag="oacc",
                                    name="out_acc")
                           if layer == L - 1 else None)
                sc_acc = (xtp.tile([128, BPC], F32, tag="oscl",
                                   name="sc_acc")
                          if layer == L - 1 else None)

                for j in range(C):
                    t, jk = divmod(j, K)
                    # group gathers + batched edge math (issued at first use)
                    g = j // G_F
                    if g not in feat_tiles:
                        g0 = g * G_F
                        gn = min(G_F, C - g0)
                        ft = featp.tile([128, gn, XW_STRIDE], BF16, tag="ft")
                        nc.gpsimd.dma_gather(
                            out_ap=ft[:],
                            in_ap=xw_hbm[:, :],
                            idxs_ap=fi_sb[:, g0 * 8:(g0 + gn) * 8],
                            num_idxs=gn * 128,
                            num_idxs_reg=gn * 128,
                            elem_size=XW_STRIDE,
                        )
                        feat_tiles[g] = ft
                        st = sidep.tile([128, gn, 128], BF16, tag="st")
                        nc.gpsimd.dma_gather(
                            out_ap=st[:],
                            in_ap=xw_hbm[:, 768:XW_STRIDE],
                            idxs_ap=si_sb[:, g0 * 8:(g0 + gn) * 8],
                            num_idxs=gn * 128,
                            num_idxs_reg=gn * 128,
                            elem_size=128,
                            elem_step=XW_STRIDE,
                        )
                        # z = a_src[src] + a_dst[dst]; leaky relu; exp --
                        # all gn chunks x 6 heads in one op each
                        z4 = workp.tile([128, gn, 6], F32, tag="z")
                        nc.vector.tensor_tensor(
                            out=z4[:], in0=ft[:, :, 768:774],
                            in1=st[:, :, 6:12], op=OP.add)
                        z24 = workp.tile([128, gn, 6], F32, tag="z2")
                        nc.vector.scalar_tensor_tensor(
                            out=z24[:], in0=z4[:], scalar=NEG, in1=z4[:],
                            op0=OP.mult, op1=OP.max)
                        rhs4 = workp.tile([128, gn, RW], BF16, tag="rhs")
                        nc.scalar.activation(out=rhs4[:, :, 768:774],
                                             in_=z24[:], func=AF.Exp)
                        # rhs feat cols: feat_h * ex_h, broadcast ex over c
                        mul_in0 = ft[:, :, 0:768].rearrange(
                            "p g (h c) -> p g h c", h=H)
                        mul_in1 = rhs4[:, :, 768:774].rearrange(
                            "p g (h c) -> p g h c", c=1)
                        mul_in0, mul_in1 = bass.broadcast_tensor_aps(
                            mul_in0, mul_in1)
                        nc.vector.tensor_tensor(
                            out=rhs4[:, :, 0:768].rearrange(
                                "p g (h c) -> p g h c", h=H),
                            in0=mul_in0, in1=mul_in1, op=OP.mult)
                        # one-hot rows: (col == dst_local), all gn chunks
                        Bt4 = workp.tile([128, gn, 128], BF16, tag="B")
                        oh_in0 = col_idx[:, :].rearrange(
                            "p (g c) -> p g c", g=1)
                        oh_in1 = dl_sb[:, g0:g0 + gn].rearrange(
                            "p (g c) -> p g c", c=1)
                        oh_in0, oh_in1 = bass.broadcast_tensor_aps(
                            oh_in0, oh_in1)
                        nc.vector.tensor_tensor(
                            out=Bt4[:], in0=oh_in0, in1=oh_in1,
                            op=OP.is_equal)
                        grp_tiles[g] = (rhs4, Bt4)
                    jj = j - g * G_F
                    rhs4, Bt4 = grp_tiles[g]
                    rhs = rhs4[:, jj, :]
                    Bt = Bt4[:, jj, :]

                    if jk == 0:
                        acc = accp.tile([128, RW], F32, space="PSUM",
                                        tag="acc")
                        feat_tiles[("acc", t)] = acc
                    acc = feat_tiles[("acc", t)]
                    nc.tensor.matmul(out=acc[:, 0:512], lhsT=Bt[:],
                                     rhs=rhs[:, 0:512],
                                     start=(jk == 0), stop=(jk == K - 1))
                    nc.tensor.matmul(out=acc[:, 512:RW], lhsT=Bt[:],
                                     rhs=rhs[:, 512:RW],
                                     start=(jk == 0), stop=(jk == K - 1))

                    # -------- block epilogue -------------------------------
                    if jk == K - 1:
                        dpe = workp.tile([128, 6], F32, tag="dpe")
                        nc.vector.tensor_scalar(
                            out=dpe[:], in0=acc[:, 768:774], scalar1=1e-20,
                            scalar2=None, op0=OP.add)
                        recip = workp.tile([128, 6], F32, tag="recip")
                        nc.vector.reciprocal(out=recip[:], in_=dpe[:])
                        accsb = workp.tile([128, 128], F32, tag="accsb")
                        nc.vector.tensor_scalar(
                            out=accsb[:], in0=acc[:, 0:128],
                            scalar1=recip[:, 0:1], scalar2=None, op0=OP.mult)
                        for h in range(1, H):
                            nc.vector.scalar_tensor_tensor(
                                out=accsb[:], in0=acc[:, h * 128:(h + 1) * 128],
                                scalar=recip[:, h:h + 1], in1=accsb[:],
                                op0=OP.mult, op1=OP.add)
                        res = workp.tile([128, 128], F32, tag="res")
                        nc.vector.scalar_tensor_tensor(
                            out=res[:], in0=accsb[:], scalar=1.0 / H,
                            in1=b_sb[:], op0=OP.mult, op1=OP.add)
                        nc.vector.tensor_scalar(
                            out=res[:], in0=res[:], scalar1=0.0,
                            scalar2=None, op0=OP.max)
                        if layer == L - 1:
                            rmax = workp.tile([128, 1], F32, tag="rmax")
                            nc.vector.tensor_reduce(
                                out=rmax[:], in_=res[:],
                                axis=mybir.AxisListType.X, op=OP.max)
                            rg = workp.tile([128, 1], F32, tag="rg")
                            nc.vector.tensor_scalar(
                                out=rg[:], in0=rmax[:], scalar1=1e-30,
                                scalar2=None, op0=OP.max)
                            nc.vector.tensor_copy(out=sc_acc[:, t:t + 1],
                                                  in_=rg[:])
                            rinv = workp.tile([128, 1], F32, tag="rinv")
                            nc.vector.reciprocal(out=rinv[:], in_=rg[:])
                            r255 = workp.tile([128, 1], F32, tag="r255")
                            nc.vector.tensor_scalar(
                                out=r255[:], in0=rinv[:], scalar1=255.0,
                                scalar2=None, op0=OP.mult)
                            nc.vector.tensor_scalar(
                                out=out_acc[:, t, :], in0=res[:],
                                scalar1=r255[:, 0:1], scalar2=None,
                                op0=OP.mult)
                            if t == BPC - 1:
                                nc.sync.dma_start(
                                    out=out_ext[0:SHARD, :].rearrange(
                                        "(t p) c -> p t c", p=128),
                                    in_=out_acc[:])
                                nc.sync.dma_start(
                                    out=out_ext[SHARD:SHARD + 4 * BPC, :]
                                    .rearrange("a c -> c a"),
                                    in_=sc_acc[:].bitcast(U8))
                        else:
                            pst = psdp.tile([128, 128], F32, space="PSUM",
                                            tag="psd")
                            nc.tensor.transpose(out=pst[:], in_=res[:],
                                                identity=ident[:])
                            nc.vector.tensor_copy(
                                out=xt_sb[:, t * 128:(t + 1) * 128],
                                in_=pst[:])
                            if t == BPC // 2 - 1 or t == BPC - 1:
                                h = 0 if t == BPC // 2 - 1 else 1
                                hsl = slice(h * (SHARD // 2),
                                            (h + 1) * (SHARD // 2))
                                agi = ag_ins[layer * 2 + h]
                                nc.sync.dma_start(out=agi[:, :],
                                                  in_=xt_sb[:, hsl])
                                nc.gpsimd.collective_compute(
                                    "AllGather",
                                    OP.bypass,
                                    replica_groups=[list(range(CORES))],
                                    ins=[agi.opt()],
                                    outs=[ag_outs[layer * 2 + h].opt()],
                                )

    nc.compile()
    return nc


_NC_CACHE = {}


def _get_nc(K):
    if K not in _NC_CACHE:
        nc = _build_nc(K)
        # The BIR is immutable once compiled, but the jit lowering
        # re-serializes it (~0.14 s) on every dispatch; serve a cached copy.
        bj = nc.to_json_bytes()
        nc.to_json_bytes = lambda: bj
        _NC_CACHE[K] = nc
    return _NC_CACHE[K]


def kernel(**inputs):
    x = np.asarray(inputs["x"], np.float32)
    ei = np.asarray(inputs["edge_index"], np.int64)
    Ws = [np.asarray(inputs[f"W{i}"], np.float32) for i in range(L)]
    asrcs = [np.asarray(inputs[f"att_src{i}"], np.float32) for i in range(L)]
    adsts = [np.asarray(inputs[f"att_dst{i}"], np.float32) for i in range(L)]
    bs = [np.asarray(inputs[f"b{i}"], np.float32) for i in range(L)]

    prep = _host_prep(x, ei, Ws, asrcs, adsts, bs)
    nc = _get_nc(prep["K"])

    in_maps = [{"pk": prep["pk"][s]} for s in range(CORES)]

    res = run_bass_kernel_spmd(nc, in_maps, list(range(CORES)))
    if os.environ.get("GAT_BENCH"):
        import time
        times = []
        for _ in range(int(os.environ.get("GAT_BENCH_ITERS", "14"))):
            t0 = time.time()
            res = run_bass_kernel_spmd(nc, in_maps, list(range(CORES)))
            times.append(time.time() - t0)
        kernel.bench_wall_s = times
    parts = []
    for s in range(CORES):
        raw = np.asarray(res.results[s]["out_shard"])   # [SHARD+4*BPC, 128]
        u8 = raw[:SHARD]
        rmax = np.ascontiguousarray(
            raw[SHARD:SHARD + 4 * BPC, :].T).view(np.float32)    # [128, BPC]
        # node row t*128+p within the shard has scale rmax[p, t] / 255
        scale = np.ascontiguousarray(rmax.T).reshape(SHARD, 1) / 255.0
        parts.append(u8.astype(np.float32) * scale)
    return np.concatenate(parts, axis=0)[:N]
